# revision 16
# baseline (speedup 1.0000x reference)
"""Trainium2 Bass kernel for nn_ContextualViewModel (gnn_message_passing).

Reference semantics:
    sx, sy = station_ids // 512, station_ids % 512
    s = sum_k x[sx_k, sy_k] @ W          # a single (128,) vector
    out = broadcast_to(s, (512, 512, 128))

The compute is tiny; the problem is memory-bound on writing the 128 MiB
output. Sharding: split the (i,j) grid of the output across 8 cores
(64 rows of 512 each -> 16 MiB per core). The K=128 gathered station rows
and W are replicated to every core (gathered host-side while slicing
inputs, per the sharding hint).

Per-core pipeline (tuned against perfetto/NTFF traces):
  - One packed load [gT | W] -> SBUF (one DMA, one sem wait).
  - DVE reduce_sum gives u[c] = sum_k g[k,c] (~0.2 us, no PE needed).
  - Two back-to-back float32r matmuls with a 0-stride broadcast lhsT
    write b[p,d] = s[d] into TWO separate PSUM banks. (Two banks
    because concurrent DVE+ACT reads of the SAME PSUM bank hard-fault
    the device: NRT_EXEC_UNIT_UNRECOVERABLE, measured. float32r is
    single-pass, ~2x faster than fp32 LOW_HIGH and plenty accurate.)
  - DVE and ACT copy the two halves of a [128, 2048] replicated tile in
    parallel (0-stride repeat reads of their own PSUM bank), so the
    full-width tile is ready as soon as the slower copy finishes.
  - The 16 MiB shard streams out as 16 uniform [128, 2048]-float chunk
    stores alternating between the two HWDGE queues (sync/scalar).

Store schedule notes (measured): an n-line store is split over
m = n/d engines (d = smallest divisor of n with d >= n/16) filling
engine slots from 0, and ONLY exact 128-partition stores get the
port-aligned (swizzled) partition->engine assignment. Partial-partition
stores make engines read partitions on foreign SBUF AXI ports, which
drops stream throughput ~25% in that phase (419 -> ~320 GB/s). So all
stores here are full 128-line chunks; per-engine rates are then a
uniform ~26 GB/s x 16 engines ~= 419 GB/s, the practical ceiling.
"""

import sys

import numpy as np

try:
    import concourse  # noqa: F401
except ImportError:  # pragma: no cover
    sys.path.insert(0, "/opt/trn_rl_repo")

H, WD, K = 512, 512, 128
N_CORES = 8
ROWS_PER_CORE = H // N_CORES           # 64 rows of the (i) axis per core
SHARD_FLOATS = ROWS_PER_CORE * WD * K  # 4,194,304 floats = 16 MiB

LINE_F = 2048                          # floats per line (8 KiB descriptors)
N_LINES = SHARD_FLOATS // LINE_F       # 2048 lines per core
HALF_W = LINE_F // 2
N_CHUNKS = N_LINES // 128              # 16 full-width chunk stores

_NC = None


def _build():
    """Raw bacc build: manual semaphores, no Tile scheduling overhead."""
    from contextlib import ExitStack

    import concourse.bass as bass
    import concourse.bacc as bacc
    import concourse.mybir as mybir

    f32 = mybir.dt.float32
    bf16 = mybir.dt.bfloat16
    nc = bacc.Bacc(
        "TRN2", target_bir_lowering=False, debug=False, num_devices=N_CORES
    )

    gw_dram = nc.dram_tensor("gw", [K, 2 * K], f32, kind="ExternalInput")
    out_dram = nc.dram_tensor("out", [N_LINES, LINE_F], f32, kind="ExternalOutput")

    chunks = [(c * 128, (c + 1) * 128) for c in range(N_CHUNKS)]
    sync_stores = chunks[0::2]
    scalar_stores = chunks[1::2]
    n_stores = len(chunks)

    with ExitStack() as ctx:
        ec = ctx.enter_context
        gwt = ec(nc.sbuf_tensor("gwt", [K, 2 * K], f32))
        w_bf = ec(nc.sbuf_tensor("w_bf", [K, K], bf16))
        r_sb = ec(nc.sbuf_tensor("r_sb", [K, 1], f32))
        r_bf = ec(nc.sbuf_tensor("r_bf", [K, 1], bf16))
        rep = ec(nc.sbuf_tensor("rep", [128, LINE_F], f32))
        b_ps = ec(nc.psum_tensor("b_ps", [128, K], f32))
        b_ps2 = ec(nc.psum_tensor("b_ps2", [128, K], f32))
        sem_in = ec(nc.semaphore("sem_in"))
        sem_r = ec(nc.semaphore("sem_r"))
        sem_p = ec(nc.semaphore("sem_p"))
        sem_v = ec(nc.semaphore("sem_v"))
        sem_a = ec(nc.semaphore("sem_a"))
        sem_out = ec(nc.semaphore("sem_out"))
        block = ec(nc.Block())

        # 0-stride repeat reads of PSUM: [128, K] viewed as [128, HALF_W]
        def _rep_view(ps):
            base = ps[:]
            return bass.AP(
                tensor=base.tensor,
                offset=base.offset,
                ap=[[K, 128], [0, HALF_W // K], [1, K]],
            )

        b_rep = _rep_view(b_ps)
        b_rep2 = _rep_view(b_ps2)

        @block.sync
        def _(sync):
            sync.dma_start(gwt[:], gw_dram[:]).then_inc(sem_in, 16)
            sync.wait_ge(sem_v, 1)
            sync.wait_ge(sem_a, 1)
            for ln0, ln1 in sync_stores:
                sync.dma_start(out_dram[ln0:ln1], rep[:]).then_inc(sem_out, 16)
            sync.wait_ge(sem_out, 16 * n_stores)

        @block.scalar
        def _(scalar):
            scalar.wait_ge(sem_p, 2)
            scalar.copy(rep[:, HALF_W:LINE_F], b_rep2).then_inc(sem_a, 1)
            scalar.wait_ge(sem_v, 1)
            scalar.wait_ge(sem_a, 1)
            for ln0, ln1 in scalar_stores:
                scalar.dma_start(out_dram[ln0:ln1], rep[:]).then_inc(sem_out, 16)

        @block.tensor
        def _(tensor):
            tensor.wait_ge(sem_r, 2)
            # lhsT[c, p] = u[c] via 0-stride broadcast ->
            # b[p, d] = sum_c u[c] W[c, d] = s[d] on every partition
            # bf16 operands: single-pass matmul (fp32 is 2-pass LOW_HIGH;
            # float32r is rejected by the BIR verifier for unrounded
            # producers). |rel err| ~1e-3, tolerance is 2e-2.
            r_base = r_bf[:]
            r_bc = bass.AP(
                tensor=r_base.tensor, offset=r_base.offset, ap=[[1, K], [0, K]]
            )
            w_ap = w_bf[:]
            tensor.matmul(b_ps[:], r_bc, w_ap, start=True, stop=True).then_inc(
                sem_p, 1
            )
            tensor.matmul(b_ps2[:], r_bc, w_ap, start=True, stop=True).then_inc(
                sem_p, 1
            )

        @block.vector
        def _(vector):
            vector.wait_ge(sem_in, 16)
            # u[c] = sum_k g[k, c]: free-dim row-sum of the gT half.
            # DVE is pipelined: the intra-DVE RAW on r_sb needs a sem.
            vector.reduce_sum(
                r_sb[:], gwt[:, 0:K], axis=mybir.AxisListType.X
            ).then_inc(sem_r, 1)
            vector.tensor_copy(w_bf[:], gwt[:, K : 2 * K])
            vector.wait_ge(sem_r, 1)
            vector.tensor_copy(r_bf[:], r_sb[:]).then_inc(sem_r, 1)
            vector.wait_ge(sem_p, 1)
            vector.tensor_copy(rep[:, 0:HALF_W], b_rep).then_inc(sem_v, 1)

    nc.compile()
    return nc


def _get_nc():
    global _NC
    if _NC is None:
        _NC = _build()
    return _NC


def _prep_inputs(x: np.ndarray, W: np.ndarray, station_ids: np.ndarray):
    x = np.asarray(x, dtype=np.float32)
    W = np.asarray(W, dtype=np.float32)
    sid = np.asarray(station_ids).astype(np.int64)
    sx = sid // H
    sy = sid % WD
    g = x[sx, sy]  # (K, K) gathered station rows
    gw = np.ascontiguousarray(
        np.concatenate([g.T, W], axis=1), dtype=np.float32
    )  # [K, 2K]: columns 0:K = g^T, K:2K = W
    return gw


def _run(gw: np.ndarray, trace: bool = False):
    from concourse.bass_utils import run_bass_kernel_spmd

    nc = _get_nc()
    in_maps = [{"gw": gw} for _ in range(N_CORES)]
    return run_bass_kernel_spmd(nc, in_maps, list(range(N_CORES)), trace=trace)


def kernel(x: np.ndarray, W: np.ndarray, station_ids: np.ndarray) -> np.ndarray:
    gw = _prep_inputs(x, W, station_ids)
    res = _run(gw).results
    shards = [res[c]["out"].reshape(ROWS_PER_CORE, WD, K) for c in range(N_CORES)]
    return np.concatenate(shards, axis=0)


# revision 21
# speedup vs baseline: 1.1538x; 1.1538x over previous
"""Trainium2 Bass kernel for nn_ContextualViewModel (gnn_message_passing).

Reference semantics:
    sx, sy = station_ids // 512, station_ids % 512
    s = sum_k x[sx_k, sy_k] @ W          # a single (128,) vector
    out = broadcast_to(s, (512, 512, 128))

The compute is tiny; the problem is memory-bound on writing the 128 MiB
output. Sharding: split the (i,j) grid of the output across 8 cores
(64 rows of 512 each -> 16 MiB per core). The K=128 gathered station rows
and W are replicated to every core (gathered host-side while slicing
inputs, per the sharding hint).

Per-core pipeline (tuned against perfetto/NTFF traces):
  - One packed load [gT | W] -> SBUF (one DMA, one sem wait).
  - DVE reduce_sum gives u[c] = sum_k g[k,c] (~0.2 us, no PE needed).
  - Two back-to-back float32r matmuls with a 0-stride broadcast lhsT
    write b[p,d] = s[d] into TWO separate PSUM banks. (Two banks
    because concurrent DVE+ACT reads of the SAME PSUM bank hard-fault
    the device: NRT_EXEC_UNIT_UNRECOVERABLE, measured. float32r is
    single-pass, ~2x faster than fp32 LOW_HIGH and plenty accurate.)
  - DVE and ACT copy the two halves of a [128, 2048] replicated tile in
    parallel (0-stride repeat reads of their own PSUM bank), so the
    full-width tile is ready as soon as the slower copy finishes.
  - The 16 MiB shard streams out as 16 uniform [128, 2048]-float chunk
    stores alternating between the two HWDGE queues (sync/scalar).

Store schedule notes (measured): an n-line store is split over
m = n/d engines (d = smallest divisor of n with d >= n/16) filling
engine slots from 0, and ONLY exact 128-partition stores get the
port-aligned (swizzled) partition->engine assignment. Partial-partition
stores make engines read partitions on foreign SBUF AXI ports, which
drops stream throughput ~25% in that phase (419 -> ~320 GB/s). So all
stores here are full 128-line chunks; per-engine rates are then a
uniform ~26 GB/s x 16 engines ~= 419 GB/s, the practical ceiling.
"""

import sys

import numpy as np

try:
    import concourse  # noqa: F401
except ImportError:  # pragma: no cover
    sys.path.insert(0, "/opt/trn_rl_repo")

H, WD, K = 512, 512, 128
N_CORES = 8
ROWS_PER_CORE = H // N_CORES           # 64 rows of the (i) axis per core
SHARD_FLOATS = ROWS_PER_CORE * WD * K  # 4,194,304 floats = 16 MiB

LINE_F = 2048                          # floats per line (8 KiB descriptors)
N_LINES = SHARD_FLOATS // LINE_F       # 2048 lines per core
HALF_W = LINE_F // 2
N_CHUNKS = N_LINES // 128              # 16 full-width chunk stores

_NC = None


def _build():
    """Raw bacc build: manual semaphores, no Tile scheduling overhead."""
    from contextlib import ExitStack

    import concourse.bass as bass
    import concourse.bacc as bacc
    import concourse.mybir as mybir

    f32 = mybir.dt.float32
    bf16 = mybir.dt.bfloat16
    nc = bacc.Bacc(
        "TRN2", target_bir_lowering=False, debug=False, num_devices=N_CORES
    )

    gw_dram = nc.dram_tensor("gw", [K, 2 * K], bf16, kind="ExternalInput")
    out_dram = nc.dram_tensor("out", [N_LINES, LINE_F], f32, kind="ExternalOutput")

    chunks = [(c * 128, (c + 1) * 128) for c in range(N_CHUNKS)]
    sync_stores = chunks[0::2]
    scalar_stores = chunks[1::2]
    n_stores = len(chunks)

    with ExitStack() as ctx:
        ec = ctx.enter_context
        gwt = ec(nc.sbuf_tensor("gwt", [K, 2 * K], bf16))
        r_sb = ec(nc.sbuf_tensor("r_sb", [K, 1], f32))
        r_bf = ec(nc.sbuf_tensor("r_bf", [K, 1], bf16))
        rep = ec(nc.sbuf_tensor("rep", [128, LINE_F], f32))
        b_ps = ec(nc.psum_tensor("b_ps", [128, K], f32))
        b_ps2 = ec(nc.psum_tensor("b_ps2", [128, K], f32))
        sem_in = ec(nc.semaphore("sem_in"))
        sem_r = ec(nc.semaphore("sem_r"))
        sem_p = ec(nc.semaphore("sem_p"))
        sem_v = ec(nc.semaphore("sem_v"))
        sem_a = ec(nc.semaphore("sem_a"))
        sem_out = ec(nc.semaphore("sem_out"))
        block = ec(nc.Block())

        # 0-stride repeat reads of PSUM: [128, K] viewed as [128, HALF_W]
        def _rep_view(ps):
            base = ps[:]
            return bass.AP(
                tensor=base.tensor,
                offset=base.offset,
                ap=[[K, 128], [0, HALF_W // K], [1, K]],
            )

        b_rep = _rep_view(b_ps)
        b_rep2 = _rep_view(b_ps2)

        @block.sync
        def _(sync):
            sync.dma_start(gwt[:], gw_dram[:]).then_inc(sem_in, 16)
            sync.wait_ge(sem_v, 1)
            sync.wait_ge(sem_a, 1)
            for ln0, ln1 in sync_stores:
                sync.dma_start(out_dram[ln0:ln1], rep[:]).then_inc(sem_out, 16)
            sync.wait_ge(sem_out, 16 * n_stores)

        @block.scalar
        def _(scalar):
            scalar.wait_ge(sem_p, 2)
            scalar.copy(rep[:, HALF_W:LINE_F], b_rep2).then_inc(sem_a, 1)
            scalar.wait_ge(sem_v, 1)
            scalar.wait_ge(sem_a, 1)
            for ln0, ln1 in scalar_stores:
                scalar.dma_start(out_dram[ln0:ln1], rep[:]).then_inc(sem_out, 16)

        @block.tensor
        def _(tensor):
            tensor.wait_ge(sem_r, 2)
            # lhsT[c, p] = u[c] via 0-stride broadcast ->
            # b[p, d] = sum_c u[c] W[c, d] = s[d] on every partition
            # bf16 operands: single-pass matmul (fp32 is 2-pass LOW_HIGH;
            # float32r is rejected by the BIR verifier for unrounded
            # producers). |rel err| ~1e-3, tolerance is 2e-2.
            r_base = r_bf[:]
            r_bc = bass.AP(
                tensor=r_base.tensor, offset=r_base.offset, ap=[[1, K], [0, K]]
            )
            w_ap = gwt[:, K : 2 * K]
            tensor.matmul(b_ps[:], r_bc, w_ap, start=True, stop=True).then_inc(
                sem_p, 1
            )
            tensor.matmul(b_ps2[:], r_bc, w_ap, start=True, stop=True).then_inc(
                sem_p, 1
            )

        @block.vector
        def _(vector):
            vector.wait_ge(sem_in, 16)
            # u[c] = sum_k g[k, c]: free-dim row-sum of the gT half.
            # DVE is pipelined: the intra-DVE RAW on r_sb needs a sem.
            vector.reduce_sum(
                r_sb[:], gwt[:, 0:K], axis=mybir.AxisListType.X
            ).then_inc(sem_r, 1)
            vector.wait_ge(sem_r, 1)
            vector.tensor_copy(r_bf[:], r_sb[:]).then_inc(sem_r, 1)
            vector.wait_ge(sem_p, 1)
            vector.tensor_copy(rep[:, 0:HALF_W], b_rep).then_inc(sem_v, 1)

    nc.compile()
    return nc


def _get_nc():
    global _NC
    if _NC is None:
        _NC = _build()
    return _NC


def _prep_inputs(x: np.ndarray, W: np.ndarray, station_ids: np.ndarray):
    x = np.asarray(x, dtype=np.float32)
    W = np.asarray(W, dtype=np.float32)
    sid = np.asarray(station_ids).astype(np.int64)
    sx = sid // H
    sy = sid % WD
    g = x[sx, sy]  # (K, K) gathered station rows
    import ml_dtypes

    gw = np.ascontiguousarray(
        np.concatenate([g.T, W], axis=1).astype(ml_dtypes.bfloat16)
    )  # [K, 2K] bf16: columns 0:K = g^T, K:2K = W
    return gw


def _run(gw: np.ndarray, trace: bool = False):
    from concourse.bass_utils import run_bass_kernel_spmd

    nc = _get_nc()
    in_maps = [{"gw": gw} for _ in range(N_CORES)]
    return run_bass_kernel_spmd(nc, in_maps, list(range(N_CORES)), trace=trace)


def kernel(x: np.ndarray, W: np.ndarray, station_ids: np.ndarray) -> np.ndarray:
    gw = _prep_inputs(x, W, station_ids)
    res = _run(gw).results
    shards = [res[c]["out"].reshape(ROWS_PER_CORE, WD, K) for c in range(N_CORES)]
    return np.concatenate(shards, axis=0)
